# revision 1
# baseline (speedup 1.0000x reference)
"""Full (non-causal) multi-head attention for Trainium2, 8-core SPMD.

Problem: B=4, L=2048, H=16, E=64 fp32.
  scores = einsum('blhe,bshe->bhls', Q, K) * 1/sqrt(E)
  attn   = softmax(scores, axis=-1)
  out    = einsum('bhls,bshd->blhd', attn, V)

Sharding: the 64 (b,h) pairs are split over 8 NeuronCores, 8 pairs per
core; attention is fully independent per (b,h), so no cross-core
communication.  The host hands each core Q^T/K^T already transposed
([E, L], bf16) so DMA lands them ready for the PE, and takes back an
unnormalized O'[e+1, l] per pair — the softmax denominator ride-along
row — dividing + final transpose on the host (0.1% of the FLOPs).

Per-core algorithm (per (b,h) pair):
  - DMA Q^T into both partition halves (duplicated) and K^T chunk-pairs
    split across partition halves, so the QK^T matmuls can run 64x128
    row-tiled (contraction is only E=64).
  - Scores are computed transposed, S^T[s, l], so the softmax
    normalizer and the AV matmul both contract over s on partitions.
  - exp() runs on ScalarE straight out of PSUM in [128, 1024] tiles,
    rounding to f32r (full-rate fp32) for the AV matmul.
  - AV accumulates O'[e+1, l] over s-chunks in PSUM; V carries a ones
    column so row 64 of O' is the softmax denominator.
"""

import numpy as np
import ml_dtypes
from contextlib import ExitStack

import concourse.bass as bass
import concourse.mybir as mybir
import concourse.tile as tile
from concourse import bacc
from concourse.bass_utils import run_bass_kernel_spmd

N_CORES = 8
B, L, H, E = 4, 2048, 16, 64
PAIRS = (B * H) // N_CORES    # 8 (b,h) pairs per core
P = 128                       # s-chunk size / partition count
NCHUNK = L // P               # 16 s-chunks
LQ = 512                      # l-quarter (one PSUM bank of fp32)
NPASS = L // LQ               # 4 passes over l per pair
SCALE = 1.0 / 8.0             # 1/sqrt(E)

F32 = mybir.dt.float32
F32R = mybir.dt.float32r
BF16 = mybir.dt.bfloat16

QK_BF16 = True                # QK^T in bf16 (host-cast); else f32r
AV_BF16 = False               # AV (P@V) in bf16; else f32r


def _attention(tc: tile.TileContext, o, qt, kt_d, v):
    nc = tc.nc
    EXPF = mybir.ActivationFunctionType.Exp
    qk_dt = BF16 if QK_BF16 else F32

    with ExitStack() as ctx:
        raw = ctx.enter_context(tc.tile_pool(name="raw", bufs=2))
        qk_t = ctx.enter_context(tc.tile_pool(name="qk_t", bufs=2))
        etp = ctx.enter_context(tc.tile_pool(name="etp", bufs=3))
        osb = ctx.enter_context(tc.tile_pool(name="osb", bufs=2))

        # PSUM: score 3x2 banks + oacc 2x1 = 8
        pscore = ctx.enter_context(tc.tile_pool(name="pscore", bufs=3, space="PSUM"))
        pacc = ctx.enter_context(tc.tile_pool(name="pacc", bufs=2, space="PSUM"))

        # s-chunk groups per exp call (2 chunks = FD 1024 per ScalarE call;
        # larger groups make the PE the per-group pacer and stall ScalarE)
        groups = [(c, 2) for c in range(0, NCHUNK, 2)]

        for p in range(PAIRS):
            # ---- load Q^T (duplicated to both halves), K^T (paired), V ----
            qtd = qk_t.tile([P, L], qk_dt, tag="qtd")
            nc.sync.dma_start(out=qtd[0:E, :], in_=qt[p])
            nc.sync.dma_start(out=qtd[E:P, :], in_=qt[p])

            # kt_d[p] is [2, 8, 64, 128]: half h holds chunks 2c+h.
            kt = qk_t.tile([P, NCHUNK // 2, P], qk_dt, tag="kt")
            nc.sync.dma_start(
                out=kt[0:E, :, :], in_=kt_d[p, 0].rearrange("c e l -> e c l")
            )
            nc.sync.dma_start(
                out=kt[E:P, :, :], in_=kt_d[p, 1].rearrange("c e l -> e c l")
            )
            if not QK_BF16:
                qtd = qtd.bitcast(F32R)
                kt = kt.bitcast(F32R)

            vr = raw.tile([P, NCHUNK, E + 1], F32, tag="vr")
            nc.sync.dma_start(
                out=vr[:, :, 0:E], in_=v[p].rearrange("(c p) e -> p c e", p=P)
            )
            nc.vector.memset(vr[:, :, E : E + 1], 1.0)
            if AV_BF16:
                vp = qk_t.tile([P, NCHUNK, E + 1], BF16, tag="vp")
                nc.vector.tensor_copy(vp[:], vr[:])
                vpr = vp
            else:
                vp = qk_t.tile([P, NCHUNK, E + 1], F32, tag="vp")
                nc.vector.tensor_copy(vp[:].bitcast(F32R), vr[:])
                vpr = vp.bitcast(F32R)

            # ---- main loop: scores^T -> exp -> AV ----
            osum = osb.tile([E + 1, L], F32, tag="osum")
            for lq in range(NPASS):
                oacc = pacc.tile([E + 1, LQ], F32, tag="oacc")
                qsl = slice(lq * LQ, (lq + 1) * LQ)
                for base, n in groups:
                    score = pscore.tile([P, 2 * LQ], F32, tag="score")
                    for j in range(n):
                        c = base + j
                        half = c % 2
                        lo, hi = (0, E) if half == 0 else (E, P)
                        nc.tensor.matmul(
                            score[:, j * LQ : (j + 1) * LQ],
                            kt[lo:hi, c // 2, :],
                            qtd[lo:hi, qsl],
                            start=True, stop=True,
                            tile_position=(lo, 0),
                        )
                    # exp over the whole chunk-group in one ScalarE call
                    if AV_BF16:
                        et = etp.tile([P, 2 * LQ], BF16, tag="et")
                        nc.scalar.activation(
                            et[:, 0 : n * LQ], score[:, 0 : n * LQ],
                            EXPF, scale=SCALE,
                        )
                        etr = et
                    else:
                        et = etp.tile([P, 2 * LQ], F32, tag="et")
                        nc.scalar.activation(
                            et[:, 0 : n * LQ].bitcast(F32R),
                            score[:, 0 : n * LQ],
                            EXPF, scale=SCALE,
                        )
                        etr = et.bitcast(F32R)
                    # AV accumulate: O'[e+1, l] += V'^T_chunk @ E_chunk
                    for j in range(n):
                        c = base + j
                        nc.tensor.matmul(
                            oacc[:], vpr[:, c, :], etr[:, j * LQ : (j + 1) * LQ],
                            start=(c == 0), stop=(c == NCHUNK - 1),
                        )
                nc.vector.tensor_copy(osum[:, qsl], oacc[:])

            nc.sync.dma_start(out=o[p], in_=osum[:])


_CACHE = {}


def _build():
    if "nc" in _CACHE:
        return _CACHE["nc"]
    nc = bacc.Bacc("TRN2", target_bir_lowering=False, debug=False,
                   num_devices=N_CORES)
    qk_dt = BF16 if QK_BF16 else F32
    qt = nc.dram_tensor("qt", [PAIRS, E, L], qk_dt, kind="ExternalInput").ap()
    kt = nc.dram_tensor("kt", [PAIRS, 2, NCHUNK // 2, E, P], qk_dt,
                        kind="ExternalInput").ap()
    v = nc.dram_tensor("v", [PAIRS, L, E], F32, kind="ExternalInput").ap()
    o = nc.dram_tensor("o", [PAIRS, E + 1, L], F32, kind="ExternalOutput").ap()
    with tile.TileContext(nc) as tc:
        _attention(tc, o, qt, kt, v)
    nc.compile()
    _CACHE["nc"] = nc
    return nc


def run(queries, keys, values, trace=False, **kw):
    """Run the SPMD kernel; returns (out_full, BassKernelResults)."""
    nc = _build()
    np_qk = ml_dtypes.bfloat16 if QK_BF16 else np.float32
    # [B, L, H, E] -> heads-major layouts the device DMAs straight in.
    qh = np.transpose(np.asarray(queries), (0, 2, 3, 1)).reshape(B * H, E, L)
    qh = np.ascontiguousarray(qh).astype(np_qk)         # [64, E, L]
    kh = np.transpose(np.asarray(keys), (0, 2, 3, 1)).reshape(B * H, E, L)
    # [64, E, L] -> [64, 2, 8, E, 128]: half h gets s-chunks 2c+h
    kh = kh.reshape(B * H, E, NCHUNK // 2, 2, P)
    kh = np.ascontiguousarray(np.transpose(kh, (0, 3, 2, 1, 4))).astype(np_qk)
    vh = np.transpose(np.asarray(values), (0, 2, 1, 3)).reshape(B * H, L, E)
    vh = np.ascontiguousarray(vh)
    in_maps = [
        {"qt": qh[c * PAIRS : (c + 1) * PAIRS],
         "kt": kh[c * PAIRS : (c + 1) * PAIRS],
         "v": vh[c * PAIRS : (c + 1) * PAIRS]}
        for c in range(N_CORES)
    ]
    res = run_bass_kernel_spmd(nc, in_maps, list(range(N_CORES)),
                               trace=trace, **kw)
    # [64, E+1, L]: rows 0..63 unnormalized O^T, row 64 the softmax sums
    oh = np.concatenate([res.results[c]["o"] for c in range(N_CORES)], axis=0)
    onorm = oh[:, 0:E, :] / oh[:, E : E + 1, :]          # softmax divide
    out = np.transpose(onorm.reshape(B, H, E, L), (0, 3, 1, 2))
    return np.ascontiguousarray(out), res


def kernel(queries, keys, values):
    out, _ = run(queries, keys, values)
    return out

